# revision 1
# baseline (speedup 1.0000x reference)
"""Trainium2 Bass kernel for repeated sparse COO SpMM (GNN message passing).

y <- A @ y applied LAYERS times, A[row[e], col[e]] = weights[e].
N=100000 nodes, E=3200000 edges, B=16 features, 4 layers, 8 NeuronCores.

Strategy (1D partition by destination row, per the sharding hint):
  * Host: relabel nodes so each core owns a contiguous, degree-sorted,
    load-balanced range of destinations. Bucket each core's edges into
    per-destination slots so the on-chip segment-sum is a fixed-shape
    strided reduction.
  * Gather y[col] rows with the SWDGE dma_gather instruction (vectorized
    Q7 descriptor generation, 64B per descriptor). Its indices are int16,
    so a gather window covers 32768 rows; edges are host-assigned to one
    of several overlapping windows (balanced), and windows are clustered
    into groups with a uniform per-tile slot count inside each group so
    one strided 4D-AP DVE reduce per (tile, group) sums everything.
  * y lives in DRAM with a 256B row pitch (dma_gather's stride quantum).
    After each layer: AllGather the 8 compact per-core slices, then a
    local DMA expands the compact y into the padded-pitch buffer.
"""

import numpy as np

# ---------------------------------------------------------------- problem dims
N_NODES = 100000
N_EDGES = 3200000
BATCH = 16
LAYERS = 4
NCORES = 8
P = 128
YPITCH = 64  # f32 elements per y row in DRAM (256B, dma_gather stride quantum)
WINDOW = 32768  # rows addressable by one int16-indexed gather

CHUNK_COL_BUDGET = 660  # msg-buffer columns per chunk (x64B per partition)
NGROUPS = 3
REBALANCE_PASSES = 3
NUM_QUEUES = 1


def _window_bases(npad):
    if npad <= WINDOW:
        return [0]
    # denser at the ends, where columns have only one eligible window
    cand = [0, 4096, 8192, 12544, 25088, 37632, 50176, 58000, 62720]
    bases = [b for b in cand if b < npad - WINDOW]
    bases.append(npad - WINDOW)
    return bases


class _Prep:
    """Host-side graph preprocessing, shared by kernel() and tests."""

    def __init__(self, x, weights, row, col, n_nodes, ncores, layers):
        n = n_nodes
        npc_real = n // ncores
        assert npc_real * ncores == n
        tiles = (npc_real + P - 1) // P
        npc = tiles * P
        npad = ncores * npc

        row = np.asarray(row).astype(np.int64)
        col = np.asarray(col).astype(np.int64)
        weights = np.asarray(weights, dtype=np.float32)
        deg = np.bincount(row, minlength=n)

        # ascending-degree order, snake-assigned to cores for load balance
        order = np.argsort(deg, kind="stable")
        blocks = order.reshape(npc_real, ncores).copy()
        blocks[1::2] = blocks[1::2, ::-1]
        perm = np.empty(n, dtype=np.int64)
        for c in range(ncores):
            perm[blocks[:, c]] = c * npc + np.arange(npc_real)

        new_row = perm[row]
        new_col = perm[col]

        bases = np.array(_window_bases(npad), dtype=np.int64)
        nw = len(bases)

        # --- balanced per-destination window assignment -----------------
        eorder = np.argsort(new_row, kind="stable")
        sr = new_row[eorder]
        sc = new_col[eorder]
        sw_weights = weights[eorder]
        change = np.flatnonzero(np.diff(sr)) + 1
        starts = np.concatenate(([0], change))
        counts = np.diff(np.concatenate((starts, [len(sr)])))
        dests = sr[starts]
        ndest = len(dests)
        maxdeg = int(counts.max()) if ndest else 0
        dest_ltile = (dests % npc) // P

        elig = (sc[None, :] >= bases[:, None]) & (
            sc[None, :] < bases[:, None] + WINDOW
        )  # [nw, E]

        wassign = np.zeros(len(sr), dtype=np.int64)
        loads = np.zeros((ndest, nw), dtype=np.int64)
        BIG = 1 << 30
        for r in range(maxdeg):
            sel = counts > r
            epos = starts[sel] + r
            cost = np.where(elig[:, epos].T, loads[sel], BIG)
            pick = np.argmin(cost, axis=1)
            wassign[epos] = pick
            loads[sel, pick] += 1

        for _ in range(REBALANCE_PASSES):
            d_cur = np.zeros(tiles, dtype=np.int64)
            np.maximum.at(d_cur, dest_ltile, loads.max(axis=1))
            at_max = loads == d_cur[dest_ltile][:, None]
            moved = 0
            for di in np.flatnonzero(at_max.any(axis=1) & (counts > 1)):
                wmax = int(np.argmax(loads[di]))
                lo, hi = starts[di], starts[di] + counts[di]
                mine = np.arange(lo, hi)[wassign[lo:hi] == wmax]
                if len(mine) == 0:
                    continue
                el = elig[:, mine]
                best_w, best_e = -1, -1
                best_load = loads[di, wmax] - 1
                for w in range(nw):
                    if w == wmax:
                        continue
                    ok = np.flatnonzero(el[w])
                    if len(ok) and loads[di, w] < best_load:
                        best_w, best_e, best_load = w, mine[ok[0]], loads[di, w]
                if best_w >= 0:
                    wassign[best_e] = best_w
                    loads[di, wmax] -= 1
                    loads[di, best_w] += 1
                    moved += 1
            if moved == 0:
                break

        # --- per-(tile, window) slot maxima, window grouping ------------
        dtw = np.zeros((tiles, nw), dtype=np.int64)
        for w in range(nw):
            np.maximum.at(dtw[:, w], dest_ltile, loads[:, w])
        dtw = np.maximum(dtw, 1)

        ngroups = min(NGROUPS, nw)
        sums = dtw.sum(axis=0)
        order_w = np.argsort(sums)
        import itertools

        best = None
        for cuts in itertools.combinations(range(1, nw), ngroups - 1):
            groups = np.split(order_w, list(cuts))
            tot = sum(len(g) * dtw[:, g].max(axis=1).sum() for g in groups)
            if best is None or tot < best[0]:
                best = (tot, groups)
        groups = [list(map(int, g)) for g in best[1]]

        # D per (tile, group); per-window -> group id and position in group
        dtg = np.stack(
            [dtw[:, g].max(axis=1) for g in groups], axis=1
        )  # [tiles, ngroups]
        w2g = np.zeros(nw, dtype=np.int64)
        w2pos = np.zeros(nw, dtype=np.int64)
        for gi, g in enumerate(groups):
            for pi, w in enumerate(g):
                w2g[w] = gi
                w2pos[w] = pi
        gsize = np.array([len(g) for g in groups], dtype=np.int64)

        # --- chunks of tiles by column budget ---------------------------
        colw = (dtg * gsize[None, :]).sum(axis=1)  # msg columns per tile
        chunks = []  # (t0, t1)
        t0 = 0
        while t0 < tiles:
            t1 = t0
            acc = 0
            while t1 < tiles and (t1 == t0 or acc + colw[t1] <= CHUNK_COL_BUDGET):
                acc += colw[t1]
                t1 += 1
            chunks.append((t0, t1))
            t0 = t1
        nchunks = len(chunks)
        chunk_of_tile = np.zeros(tiles, dtype=np.int64)
        for ci, (a, b) in enumerate(chunks):
            chunk_of_tile[a:b] = ci

        # per-chunk per-group widths and offsets
        wcg = np.zeros((nchunks, ngroups), dtype=np.int64)  # sum of dtg in chunk
        for ci, (a, b) in enumerate(chunks):
            wcg[ci] = dtg[a:b].sum(axis=0)
        # column base of group section within a chunk buffer
        sec_base = np.zeros((nchunks, ngroups), dtype=np.int64)
        chunk_cols = np.zeros(nchunks, dtype=np.int64)
        for ci in range(nchunks):
            acc = 0
            for gi in range(ngroups):
                sec_base[ci, gi] = acc
                acc += gsize[gi] * wcg[ci, gi]
            chunk_cols[ci] = acc
        chunk_col_base = np.zeros(nchunks, dtype=np.int64)
        chunk_col_base[1:] = np.cumsum(chunk_cols)[:-1]
        total_cols = int(chunk_cols.sum())

        # tile offsets within (chunk, group): cumsum of dtg over chunk tiles
        offg = np.zeros((tiles, ngroups), dtype=np.int64)
        for ci, (a, b) in enumerate(chunks):
            offg[a:b] = np.cumsum(dtg[a:b], axis=0) - dtg[a:b]

        # --- per-edge slot index within its (dest, window) bucket -------
        grp_key = np.repeat(np.arange(ndest), counts) * nw + wassign
        gorder = np.argsort(grp_key, kind="stable")
        gs = grp_key[gorder]
        gchange = np.flatnonzero(np.diff(gs)) + 1
        gstarts = np.concatenate(([0], gchange))
        gcounts = np.diff(np.concatenate((gstarts, [len(gs)])))
        grun = np.repeat(np.arange(len(gstarts)), gcounts)
        j_sorted = np.arange(len(gs)) - gstarts[grun]
        j = np.empty(len(gs), dtype=np.int64)
        j[gorder] = j_sorted

        # --- per-edge column in the global w_s layout -------------------
        e_core = np.repeat(dests // npc, counts)
        e_ltile = np.repeat(dest_ltile, counts)
        e_p = np.repeat(dests % npc, counts) % P
        e_chunk = chunk_of_tile[e_ltile]
        e_g = w2g[wassign]
        e_wpos = w2pos[wassign]
        e_col = (
            chunk_col_base[e_chunk]
            + sec_base[e_chunk, e_g]
            + e_wpos * wcg[e_chunk, e_g]
            + offg[e_ltile, e_g]
            + j
        )

        w_all = np.zeros((ncores, P, total_cols), dtype=np.float32)
        w_all[e_core, e_p, e_col] = sw_weights

        # --- idx16 tables, one per (chunk, group, window-in-group) ------
        call_meta = []  # (chunk, group, wpos, window, entry_base, width)
        call_base = np.zeros((nchunks, nw), dtype=np.int64)  # by (chunk, w)
        acc2 = 0
        for ci in range(nchunks):
            for gi, g in enumerate(groups):
                for pi, w in enumerate(g):
                    call_base[ci, w] = acc2
                    call_meta.append(
                        (ci, gi, pi, w, int(acc2), int(wcg[ci, gi]))
                    )
                    acc2 += P * int(wcg[ci, gi])
        total_entries = int(acc2)

        val = sc - bases[wassign]
        assert (val >= 0).all() and (val < WINDOW).all()
        g_pos = (offg[e_ltile, e_g] + j) * P + e_p
        e_entry = call_base[e_chunk, wassign] + g_pos
        flat_idx = np.zeros((ncores, total_entries), dtype=np.int16)
        flat_idx[e_core, e_entry] = val.astype(np.int16)
        assert total_entries % 16 == 0
        wrapped = flat_idx.reshape(ncores, total_entries // 16, 16).transpose(
            0, 2, 1
        )
        idx16_all = np.ascontiguousarray(np.tile(wrapped, (1, 8, 1)))

        xpad = np.zeros((npad, YPITCH), dtype=np.float32)
        xpad[perm, :BATCH] = np.asarray(x, dtype=np.float32)

        self.n_nodes = n
        self.ncores = ncores
        self.layers = layers
        self.tiles = tiles
        self.npc = npc
        self.npad = npad
        self.nw = nw
        self.ngroups = ngroups
        self.gsize = [int(v) for v in gsize]
        self.bases = [int(b) for b in bases]
        self.chunks = chunks
        self.wcg = wcg
        self.sec_base = sec_base
        self.chunk_cols = chunk_cols
        self.chunk_col_base = chunk_col_base
        self.dtg = dtg
        self.offg = offg
        self.call_meta = call_meta
        self.total_cols = total_cols
        self.total_entries = total_entries
        self.perm = perm
        self.w_all = w_all
        self.idx16_all = idx16_all
        self.xpad = xpad
        self.slots = int(total_cols) * P


_REG_CACHE = {}


def _dma_gather_raw(
    g, out_ap, in_ap, idxs_ap, num_idxs, elem_size, elem_step, queue_num
):
    """Non-transpose DRAM-source dma_gather without the (transpose-only)
    elem_size%256 restriction. Mirrors bass.BassGpSimd.dma_gather."""
    import concourse.mybir as mybir

    stride_bytes = elem_step * mybir.dt.size(in_ap.dtype)
    assert stride_bytes % 256 == 0
    _in_ap = g.lower_ap_dma(in_ap, for_custom_bir_dma=True)
    _idxs_ap = g.lower_ap(idxs_ap)
    _out_ap = g.lower_ap(out_ap)
    key = (id(g), num_idxs)
    if key not in _REG_CACHE:
        _REG_CACHE[key] = g.to_reg(num_idxs)
    return g.add_instruction(
        mybir.InstDMAGatherAnt(
            name=g.bass.get_next_instruction_name(),
            ins=[*_in_ap, _idxs_ap, g.lower_val_access(_REG_CACHE[key])],
            outs=[_out_ap],
            transpose=False,
            num_idxs=num_idxs,
            elem_size=elem_size,
            stride_bytes_256=stride_bytes // 256,
            gen_mode=0,
            single_packet=True,
            queue_num=queue_num,
            sbuf_tokens_per_rank=0,
            sbuf_free_dim_per_rank=0,
            sbuf_free_dim_pad_per_rank=0,
            sbuf_byte_offset=0,
        )
    )


def build_program(prep):
    import concourse.bass as bass
    import concourse.bacc as bacc
    import concourse.mybir as mybir
    import concourse.tile as tile

    ncores = prep.ncores
    npc = prep.npc
    npad = prep.npad
    layers = prep.layers
    ngroups = prep.ngroups

    nc = bacc.Bacc(
        None,
        num_devices=ncores,
        num_swdge_queues=NUM_QUEUES,
    )
    f32 = mybir.dt.float32
    i16 = mybir.dt.int16
    xfull = nc.dram_tensor("xfull", [npad, YPITCH], f32, kind="ExternalInput")
    idx_d = nc.dram_tensor(
        "idx", [P, prep.total_entries // 16], i16, kind="ExternalInput"
    )
    w_d = nc.dram_tensor("w", [P, prep.total_cols], f32, kind="ExternalInput")
    yout = nc.dram_tensor("yout", [npc, BATCH], f32, kind="ExternalOutput")

    with tile.TileContext(nc) as tc:
        with (
            tc.tile_pool(name="res", bufs=1) as res_pool,
            tc.tile_pool(name="msgp", bufs=2) as msg_pool,
            tc.tile_pool(name="idxp", bufs=3) as idx_pool,
            tc.tile_pool(name="outp", bufs=1) as out_pool,
            tc.tile_pool(name="dram", bufs=1, space="DRAM") as dram_pool,
        ):
            w_s = res_pool.tile([P, prep.total_cols], f32, name="w_s")
            nc.sync.dma_start(out=w_s[:], in_=w_d[:])
            # prime the DVE dependency on the w_s load
            w_prime = res_pool.tile([P, 1], f32, name="w_prime")
            nc.vector.tensor_copy(out=w_prime[:], in_=w_s[:, 0:1])

            slices = [
                dram_pool.tile([npc, BATCH], f32, name=f"slice{i}")
                for i in range(max(layers - 1, 1))
            ]
            ags = [
                dram_pool.tile(
                    [npad, BATCH], f32, addr_space="Shared", name=f"ag{i}"
                )
                for i in range(max(layers - 1, 1))
            ]
            ypads = [
                dram_pool.tile([npad, YPITCH], f32, name=f"ypad{i}")
                for i in range(max(layers - 1, 1))
            ]

            qn = 0
            for l in range(layers):
                src = xfull if l == 0 else ypads[l - 1]
                dst = yout if l == layers - 1 else slices[l]
                ylayer = out_pool.tile(
                    [P, prep.tiles * BATCH], f32, name=f"ylayer{l}", tag=f"yl{l}"
                )
                for ci, (t0, t1) in enumerate(prep.chunks):
                    ccols = int(prep.chunk_cols[ci])
                    msg = msg_pool.tile(
                        [P, ccols * BATCH], f32, name="msg", tag="msg"
                    )
                    # Q7 dma_gather scratch holds ~16k int32 idxs; stay under
                    max_cols = 8
                    for (ci2, gi, pi, w, ebase, width) in prep.call_meta:
                        if ci2 != ci:
                            continue
                        base_w = prep.bases[w]
                        sec0 = int(prep.sec_base[ci, gi]) + pi * width
                        for s0 in range(0, width, max_cols):
                            sw = min(max_cols, width - s0)
                            n_idx = P * sw
                            eb = ebase + P * s0
                            cstart = (sec0 + s0) * BATCH
                            idxt = idx_pool.tile(
                                [P, n_idx // 16], i16, name="idxt", tag="idxt"
                            )
                            nc.sync.dma_start(
                                out=idxt[:],
                                in_=idx_d[:, eb // 16 : (eb + n_idx) // 16],
                            )
                            _dma_gather_raw(
                                nc.gpsimd,
                                out_ap=msg[
                                    :, cstart : cstart + sw * BATCH
                                ].rearrange("p (c f) -> p c f", f=BATCH),
                                in_ap=src[
                                    base_w : min(base_w + WINDOW, npad),
                                    0:BATCH,
                                ],
                                idxs_ap=idxt[:],
                                num_idxs=n_idx,
                                elem_size=BATCH,
                                elem_step=YPITCH,
                                queue_num=qn,
                            )
                            qn = (qn + 1) % NUM_QUEUES
                    cb = int(prep.chunk_col_base[ci])
                    nc.vector.tensor_tensor(
                        out=msg[:].rearrange("p (d f) -> p d f", f=BATCH),
                        in0=msg[:].rearrange("p (d f) -> p d f", f=BATCH),
                        in1=w_s[:, cb : cb + ccols].to_broadcast(
                            [P, ccols, BATCH]
                        ),
                        op=mybir.AluOpType.mult,
                    )
                    for t in range(t0, t1):
                        for gi in range(ngroups):
                            d_tg = int(prep.dtg[t, gi])
                            gs = prep.gsize[gi]
                            o = (
                                int(prep.sec_base[ci, gi])
                                + int(prep.offg[t, gi])
                            ) * BATCH
                            base2 = msg[:, o : o + BATCH]
                            in_ap = bass.AP(
                                base2.tensor,
                                base2.offset,
                                [
                                    base2.ap[0],
                                    [1, BATCH],
                                    [int(prep.wcg[ci, gi]) * BATCH, gs],
                                    [BATCH, d_tg],
                                ],
                            )
                            if gi == 0:
                                nc.vector.tensor_reduce(
                                    out=ylayer[:, t * BATCH : (t + 1) * BATCH],
                                    in_=in_ap,
                                    axis=mybir.AxisListType.XY,
                                    op=mybir.AluOpType.add,
                                )
                            else:
                                tmp = out_pool.tile(
                                    [P, BATCH], f32, name="tmp", tag="tmp",
                                    bufs=4,
                                )
                                nc.vector.tensor_reduce(
                                    out=tmp[:],
                                    in_=in_ap,
                                    axis=mybir.AxisListType.XY,
                                    op=mybir.AluOpType.add,
                                )
                                nc.vector.tensor_add(
                                    out=ylayer[:, t * BATCH : (t + 1) * BATCH],
                                    in0=ylayer[:, t * BATCH : (t + 1) * BATCH],
                                    in1=tmp[:],
                                )
                nc.sync.dma_start(
                    out=dst[:, :].rearrange("(t p) f -> p t f", p=P),
                    in_=ylayer[:].rearrange("p (t f) -> p t f", f=BATCH),
                )
                if l < layers - 1:
                    nc.gpsimd.collective_compute(
                        "AllGather",
                        mybir.AluOpType.bypass,
                        replica_groups=[list(range(ncores))],
                        ins=[slices[l][:]],
                        outs=[ags[l][:]],
                    )
                    # expand compact y into the 256B-pitch gather buffer
                    # (pieces keep AP dims under the 16-bit ISA field)
                    npiece = 4
                    step = npad // npiece
                    for pi2 in range(npiece):
                        a = pi2 * step
                        b = npad if pi2 == npiece - 1 else a + step
                        nc.sync.dma_start(
                            out=ypads[l][a:b, 0:BATCH], in_=ags[l][a:b, :]
                        )
    nc.compile()
    return nc


def run(prep, trace=False):
    from concourse.bass_utils import run_bass_kernel_spmd

    nc = build_program(prep)
    in_maps = [
        {"xfull": prep.xpad, "idx": prep.idx16_all[k], "w": prep.w_all[k]}
        for k in range(prep.ncores)
    ]
    res = run_bass_kernel_spmd(
        nc, in_maps, core_ids=list(range(prep.ncores)), trace=trace
    )
    y_concat = np.concatenate(
        [res.results[k]["yout"] for k in range(prep.ncores)], axis=0
    )
    return y_concat[prep.perm], res


def kernel(x, weights, row, col):
    prep = _Prep(x, weights, row, col, N_NODES, NCORES, LAYERS)
    y, _ = run(prep, trace=False)
    return y



# revision 2
# speedup vs baseline: 5.3743x; 5.3743x over previous
"""Trainium2 Bass kernel for repeated sparse COO SpMM (GNN message passing).

y <- A @ y applied LAYERS times, A[row[e], col[e]] = weights[e].
N=100000 nodes, E=3200000 edges, B=16 features, 4 layers, 8 NeuronCores.

Strategy (1D partition by destination row, per the sharding hint):
  * Host: relabel nodes so each core owns a contiguous, degree-sorted,
    load-balanced range of destinations. Bucket each core's edges into
    per-destination slots so the on-chip segment-sum is a fixed-shape
    strided reduction.
  * Gather y[col] rows with the SWDGE dma_gather instruction (vectorized
    Q7 descriptor generation, 64B per descriptor). Its indices are int16,
    so a gather window covers 32768 rows; edges are host-assigned to one
    of several overlapping windows (balanced), and windows are clustered
    into groups with a uniform per-tile slot count inside each group so
    one strided 4D-AP DVE reduce per (tile, group) sums everything.
  * y lives in DRAM with a 256B row pitch (dma_gather's stride quantum).
    After each layer: AllGather the 8 compact per-core slices, then a
    local DMA expands the compact y into the padded-pitch buffer.
"""

import numpy as np

# ---------------------------------------------------------------- problem dims
N_NODES = 100000
N_EDGES = 3200000
BATCH = 16
LAYERS = 4
NCORES = 8
P = 128
YPITCH = 64  # f32 elements per y row in DRAM (256B, dma_gather stride quantum)
WINDOW = 32768  # rows addressable by one int16-indexed gather

CHUNK_COL_BUDGET = 660  # msg-buffer columns per chunk (x64B per partition)
NGROUPS = 3
REBALANCE_PASSES = 3
NUM_QUEUES = 4


def _window_bases(npad):
    if npad <= WINDOW:
        return [0]
    # denser at the ends, where columns have only one eligible window
    cand = [0, 4096, 8192, 12544, 25088, 37632, 50176, 58000, 62720]
    bases = [b for b in cand if b < npad - WINDOW]
    bases.append(npad - WINDOW)
    return bases


class _Prep:
    """Host-side graph preprocessing, shared by kernel() and tests."""

    def __init__(self, x, weights, row, col, n_nodes, ncores, layers):
        n = n_nodes
        npc_real = n // ncores
        assert npc_real * ncores == n
        tiles = (npc_real + P - 1) // P
        npc = tiles * P
        npad = ncores * npc

        row = np.asarray(row).astype(np.int64)
        col = np.asarray(col).astype(np.int64)
        weights = np.asarray(weights, dtype=np.float32)
        deg = np.bincount(row, minlength=n)

        # ascending-degree order, snake-assigned to cores for load balance
        order = np.argsort(deg, kind="stable")
        blocks = order.reshape(npc_real, ncores).copy()
        blocks[1::2] = blocks[1::2, ::-1]
        perm = np.empty(n, dtype=np.int64)
        for c in range(ncores):
            perm[blocks[:, c]] = c * npc + np.arange(npc_real)

        new_row = perm[row]
        new_col = perm[col]

        bases = np.array(_window_bases(npad), dtype=np.int64)
        nw = len(bases)

        # --- balanced per-destination window assignment -----------------
        eorder = np.argsort(new_row, kind="stable")
        sr = new_row[eorder]
        sc = new_col[eorder]
        sw_weights = weights[eorder]
        change = np.flatnonzero(np.diff(sr)) + 1
        starts = np.concatenate(([0], change))
        counts = np.diff(np.concatenate((starts, [len(sr)])))
        dests = sr[starts]
        ndest = len(dests)
        maxdeg = int(counts.max()) if ndest else 0
        dest_ltile = (dests % npc) // P

        elig = (sc[None, :] >= bases[:, None]) & (
            sc[None, :] < bases[:, None] + WINDOW
        )  # [nw, E]

        wassign = np.zeros(len(sr), dtype=np.int64)
        loads = np.zeros((ndest, nw), dtype=np.int64)
        BIG = 1 << 30
        for r in range(maxdeg):
            sel = counts > r
            epos = starts[sel] + r
            cost = np.where(elig[:, epos].T, loads[sel], BIG)
            pick = np.argmin(cost, axis=1)
            wassign[epos] = pick
            loads[sel, pick] += 1

        for _ in range(REBALANCE_PASSES):
            d_cur = np.zeros(tiles, dtype=np.int64)
            np.maximum.at(d_cur, dest_ltile, loads.max(axis=1))
            at_max = loads == d_cur[dest_ltile][:, None]
            moved = 0
            for di in np.flatnonzero(at_max.any(axis=1) & (counts > 1)):
                wmax = int(np.argmax(loads[di]))
                lo, hi = starts[di], starts[di] + counts[di]
                mine = np.arange(lo, hi)[wassign[lo:hi] == wmax]
                if len(mine) == 0:
                    continue
                el = elig[:, mine]
                best_w, best_e = -1, -1
                best_load = loads[di, wmax] - 1
                for w in range(nw):
                    if w == wmax:
                        continue
                    ok = np.flatnonzero(el[w])
                    if len(ok) and loads[di, w] < best_load:
                        best_w, best_e, best_load = w, mine[ok[0]], loads[di, w]
                if best_w >= 0:
                    wassign[best_e] = best_w
                    loads[di, wmax] -= 1
                    loads[di, best_w] += 1
                    moved += 1
            if moved == 0:
                break

        # --- per-(tile, window) slot maxima, window grouping ------------
        dtw = np.zeros((tiles, nw), dtype=np.int64)
        for w in range(nw):
            np.maximum.at(dtw[:, w], dest_ltile, loads[:, w])
        dtw = np.maximum(dtw, 1)

        ngroups = min(NGROUPS, nw)
        sums = dtw.sum(axis=0)
        order_w = np.argsort(sums)
        import itertools

        best = None
        for cuts in itertools.combinations(range(1, nw), ngroups - 1):
            groups = np.split(order_w, list(cuts))
            tot = sum(len(g) * dtw[:, g].max(axis=1).sum() for g in groups)
            if best is None or tot < best[0]:
                best = (tot, groups)
        groups = [list(map(int, g)) for g in best[1]]

        # D per (tile, group); per-window -> group id and position in group
        dtg = np.stack(
            [dtw[:, g].max(axis=1) for g in groups], axis=1
        )  # [tiles, ngroups]
        w2g = np.zeros(nw, dtype=np.int64)
        w2pos = np.zeros(nw, dtype=np.int64)
        for gi, g in enumerate(groups):
            for pi, w in enumerate(g):
                w2g[w] = gi
                w2pos[w] = pi
        gsize = np.array([len(g) for g in groups], dtype=np.int64)

        # --- chunks of tiles by column budget ---------------------------
        colw = (dtg * gsize[None, :]).sum(axis=1)  # msg columns per tile
        chunks = []  # (t0, t1)
        t0 = 0
        while t0 < tiles:
            t1 = t0
            acc = 0
            while t1 < tiles and (t1 == t0 or acc + colw[t1] <= CHUNK_COL_BUDGET):
                acc += colw[t1]
                t1 += 1
            chunks.append((t0, t1))
            t0 = t1
        nchunks = len(chunks)
        chunk_of_tile = np.zeros(tiles, dtype=np.int64)
        for ci, (a, b) in enumerate(chunks):
            chunk_of_tile[a:b] = ci

        # per-chunk per-group widths and offsets
        wcg = np.zeros((nchunks, ngroups), dtype=np.int64)  # sum of dtg in chunk
        for ci, (a, b) in enumerate(chunks):
            wcg[ci] = dtg[a:b].sum(axis=0)
        # column base of group section within a chunk buffer
        sec_base = np.zeros((nchunks, ngroups), dtype=np.int64)
        chunk_cols = np.zeros(nchunks, dtype=np.int64)
        for ci in range(nchunks):
            acc = 0
            for gi in range(ngroups):
                sec_base[ci, gi] = acc
                acc += gsize[gi] * wcg[ci, gi]
            chunk_cols[ci] = acc
        chunk_col_base = np.zeros(nchunks, dtype=np.int64)
        chunk_col_base[1:] = np.cumsum(chunk_cols)[:-1]
        total_cols = int(chunk_cols.sum())

        # tile offsets within (chunk, group): cumsum of dtg over chunk tiles
        offg = np.zeros((tiles, ngroups), dtype=np.int64)
        for ci, (a, b) in enumerate(chunks):
            offg[a:b] = np.cumsum(dtg[a:b], axis=0) - dtg[a:b]

        # --- per-edge slot index within its (dest, window) bucket -------
        grp_key = np.repeat(np.arange(ndest), counts) * nw + wassign
        gorder = np.argsort(grp_key, kind="stable")
        gs = grp_key[gorder]
        gchange = np.flatnonzero(np.diff(gs)) + 1
        gstarts = np.concatenate(([0], gchange))
        gcounts = np.diff(np.concatenate((gstarts, [len(gs)])))
        grun = np.repeat(np.arange(len(gstarts)), gcounts)
        j_sorted = np.arange(len(gs)) - gstarts[grun]
        j = np.empty(len(gs), dtype=np.int64)
        j[gorder] = j_sorted

        # --- per-edge column in the global w_s layout -------------------
        e_core = np.repeat(dests // npc, counts)
        e_ltile = np.repeat(dest_ltile, counts)
        e_p = np.repeat(dests % npc, counts) % P
        e_chunk = chunk_of_tile[e_ltile]
        e_g = w2g[wassign]
        e_wpos = w2pos[wassign]
        e_col = (
            chunk_col_base[e_chunk]
            + sec_base[e_chunk, e_g]
            + e_wpos * wcg[e_chunk, e_g]
            + offg[e_ltile, e_g]
            + j
        )

        w_all = np.zeros((ncores, P, total_cols), dtype=np.float32)
        w_all[e_core, e_p, e_col] = sw_weights

        # --- idx16 tables, one per (chunk, group, window-in-group) ------
        call_meta = []  # (chunk, group, wpos, window, entry_base, width)
        call_base = np.zeros((nchunks, nw), dtype=np.int64)  # by (chunk, w)
        acc2 = 0
        for ci in range(nchunks):
            for gi, g in enumerate(groups):
                for pi, w in enumerate(g):
                    call_base[ci, w] = acc2
                    call_meta.append(
                        (ci, gi, pi, w, int(acc2), int(wcg[ci, gi]))
                    )
                    acc2 += P * int(wcg[ci, gi])
        total_entries = int(acc2)

        val = sc - bases[wassign]
        assert (val >= 0).all() and (val < WINDOW).all()
        g_pos = (offg[e_ltile, e_g] + j) * P + e_p
        e_entry = call_base[e_chunk, wassign] + g_pos
        flat_idx = np.zeros((ncores, total_entries), dtype=np.int16)
        flat_idx[e_core, e_entry] = val.astype(np.int16)
        assert total_entries % 16 == 0
        wrapped = flat_idx.reshape(ncores, total_entries // 16, 16).transpose(
            0, 2, 1
        )
        idx16_all = np.ascontiguousarray(np.tile(wrapped, (1, 8, 1)))

        xpad = np.zeros((npad, YPITCH), dtype=np.float32)
        xpad[perm, :BATCH] = np.asarray(x, dtype=np.float32)

        self.n_nodes = n
        self.ncores = ncores
        self.layers = layers
        self.tiles = tiles
        self.npc = npc
        self.npad = npad
        self.nw = nw
        self.ngroups = ngroups
        self.gsize = [int(v) for v in gsize]
        self.bases = [int(b) for b in bases]
        self.chunks = chunks
        self.wcg = wcg
        self.sec_base = sec_base
        self.chunk_cols = chunk_cols
        self.chunk_col_base = chunk_col_base
        self.dtg = dtg
        self.offg = offg
        self.call_meta = call_meta
        self.total_cols = total_cols
        self.total_entries = total_entries
        self.perm = perm
        self.w_all = w_all
        self.idx16_all = idx16_all
        self.xpad = xpad
        self.slots = int(total_cols) * P


_REG_CACHE = {}


def _dma_gather_raw(
    g, out_ap, in_ap, idxs_ap, num_idxs, elem_size, elem_step, queue_num
):
    """Non-transpose DRAM-source dma_gather without the (transpose-only)
    elem_size%256 restriction. Mirrors bass.BassGpSimd.dma_gather."""
    import concourse.mybir as mybir

    stride_bytes = elem_step * mybir.dt.size(in_ap.dtype)
    assert stride_bytes % 256 == 0
    _in_ap = g.lower_ap_dma(in_ap, for_custom_bir_dma=True)
    _idxs_ap = g.lower_ap(idxs_ap)
    _out_ap = g.lower_ap(out_ap)
    key = (id(g), num_idxs)
    if key not in _REG_CACHE:
        _REG_CACHE[key] = g.to_reg(num_idxs)
    return g.add_instruction(
        mybir.InstDMAGatherAnt(
            name=g.bass.get_next_instruction_name(),
            ins=[*_in_ap, _idxs_ap, g.lower_val_access(_REG_CACHE[key])],
            outs=[_out_ap],
            transpose=False,
            num_idxs=num_idxs,
            elem_size=elem_size,
            stride_bytes_256=stride_bytes // 256,
            gen_mode=0,
            single_packet=True,
            queue_num=queue_num,
            sbuf_tokens_per_rank=0,
            sbuf_free_dim_per_rank=0,
            sbuf_free_dim_pad_per_rank=0,
            sbuf_byte_offset=0,
        )
    )


def build_program(prep):
    import concourse.bass as bass
    import concourse.bacc as bacc
    import concourse.mybir as mybir
    import concourse.tile as tile

    ncores = prep.ncores
    npc = prep.npc
    npad = prep.npad
    layers = prep.layers
    ngroups = prep.ngroups

    nc = bacc.Bacc(
        None,
        num_devices=ncores,
        num_swdge_queues=NUM_QUEUES,
    )
    f32 = mybir.dt.float32
    i16 = mybir.dt.int16
    xfull = nc.dram_tensor("xfull", [npad, YPITCH], f32, kind="ExternalInput")
    idx_d = nc.dram_tensor(
        "idx", [P, prep.total_entries // 16], i16, kind="ExternalInput"
    )
    w_d = nc.dram_tensor("w", [P, prep.total_cols], f32, kind="ExternalInput")
    yout = nc.dram_tensor("yout", [npc, BATCH], f32, kind="ExternalOutput")

    with tile.TileContext(nc) as tc:
        with (
            tc.tile_pool(name="res", bufs=1) as res_pool,
            tc.tile_pool(name="msgp", bufs=2) as msg_pool,
            tc.tile_pool(name="idxp", bufs=3) as idx_pool,
            tc.tile_pool(name="outp", bufs=1) as out_pool,
            tc.tile_pool(name="dram", bufs=1, space="DRAM") as dram_pool,
        ):
            w_s = res_pool.tile([P, prep.total_cols], f32, name="w_s")
            nc.sync.dma_start(out=w_s[:], in_=w_d[:])
            # prime the DVE dependency on the w_s load
            w_prime = res_pool.tile([P, 1], f32, name="w_prime")
            nc.vector.tensor_copy(out=w_prime[:], in_=w_s[:, 0:1])

            slices = [
                dram_pool.tile([npc, BATCH], f32, name=f"slice{i}")
                for i in range(max(layers - 1, 1))
            ]
            ags = [
                dram_pool.tile(
                    [npad, BATCH], f32, addr_space="Shared", name=f"ag{i}"
                )
                for i in range(max(layers - 1, 1))
            ]
            ypads = [
                dram_pool.tile([npad, YPITCH], f32, name=f"ypad{i}")
                for i in range(max(layers - 1, 1))
            ]

            qn = 0
            for l in range(layers):
                src = xfull if l == 0 else ypads[l - 1]
                dst = yout if l == layers - 1 else slices[l]
                ylayer = out_pool.tile(
                    [P, prep.tiles * BATCH], f32, name=f"ylayer{l}", tag=f"yl{l}"
                )
                for ci, (t0, t1) in enumerate(prep.chunks):
                    ccols = int(prep.chunk_cols[ci])
                    msg = msg_pool.tile(
                        [P, ccols * BATCH], f32, name="msg", tag="msg"
                    )
                    # Q7 dma_gather scratch holds ~16k int32 idxs; stay under
                    max_cols = 8
                    for (ci2, gi, pi, w, ebase, width) in prep.call_meta:
                        if ci2 != ci:
                            continue
                        base_w = prep.bases[w]
                        sec0 = int(prep.sec_base[ci, gi]) + pi * width
                        for s0 in range(0, width, max_cols):
                            sw = min(max_cols, width - s0)
                            n_idx = P * sw
                            eb = ebase + P * s0
                            cstart = (sec0 + s0) * BATCH
                            idxt = idx_pool.tile(
                                [P, n_idx // 16], i16, name="idxt", tag="idxt"
                            )
                            nc.sync.dma_start(
                                out=idxt[:],
                                in_=idx_d[:, eb // 16 : (eb + n_idx) // 16],
                            )
                            _dma_gather_raw(
                                nc.gpsimd,
                                out_ap=msg[
                                    :, cstart : cstart + sw * BATCH
                                ].rearrange("p (c f) -> p c f", f=BATCH),
                                in_ap=src[
                                    base_w : min(base_w + WINDOW, npad),
                                    0:BATCH,
                                ],
                                idxs_ap=idxt[:],
                                num_idxs=n_idx,
                                elem_size=BATCH,
                                elem_step=YPITCH,
                                queue_num=qn,
                            )
                            qn = (qn + 1) % NUM_QUEUES
                    cb = int(prep.chunk_col_base[ci])
                    nc.vector.tensor_tensor(
                        out=msg[:].rearrange("p (d f) -> p d f", f=BATCH),
                        in0=msg[:].rearrange("p (d f) -> p d f", f=BATCH),
                        in1=w_s[:, cb : cb + ccols].to_broadcast(
                            [P, ccols, BATCH]
                        ),
                        op=mybir.AluOpType.mult,
                    )
                    for t in range(t0, t1):
                        for gi in range(ngroups):
                            d_tg = int(prep.dtg[t, gi])
                            gs = prep.gsize[gi]
                            o = (
                                int(prep.sec_base[ci, gi])
                                + int(prep.offg[t, gi])
                            ) * BATCH
                            base2 = msg[:, o : o + BATCH]
                            in_ap = bass.AP(
                                base2.tensor,
                                base2.offset,
                                [
                                    base2.ap[0],
                                    [1, BATCH],
                                    [int(prep.wcg[ci, gi]) * BATCH, gs],
                                    [BATCH, d_tg],
                                ],
                            )
                            if gi == 0:
                                nc.vector.tensor_reduce(
                                    out=ylayer[:, t * BATCH : (t + 1) * BATCH],
                                    in_=in_ap,
                                    axis=mybir.AxisListType.XY,
                                    op=mybir.AluOpType.add,
                                )
                            else:
                                tmp = out_pool.tile(
                                    [P, BATCH], f32, name="tmp", tag="tmp",
                                    bufs=4,
                                )
                                nc.vector.tensor_reduce(
                                    out=tmp[:],
                                    in_=in_ap,
                                    axis=mybir.AxisListType.XY,
                                    op=mybir.AluOpType.add,
                                )
                                nc.vector.tensor_add(
                                    out=ylayer[:, t * BATCH : (t + 1) * BATCH],
                                    in0=ylayer[:, t * BATCH : (t + 1) * BATCH],
                                    in1=tmp[:],
                                )
                nc.sync.dma_start(
                    out=dst[:, :].rearrange("(t p) f -> p t f", p=P),
                    in_=ylayer[:].rearrange("p (t f) -> p t f", f=BATCH),
                )
                if l < layers - 1:
                    nc.gpsimd.collective_compute(
                        "AllGather",
                        mybir.AluOpType.bypass,
                        replica_groups=[list(range(ncores))],
                        ins=[slices[l][:]],
                        outs=[ags[l][:]],
                    )
                    # expand compact y into the 256B-pitch gather buffer
                    # (pieces keep AP dims under the 16-bit ISA field)
                    npiece = 4
                    step = npad // npiece
                    for pi2 in range(npiece):
                        a = pi2 * step
                        b = npad if pi2 == npiece - 1 else a + step
                        nc.sync.dma_start(
                            out=ypads[l][a:b, 0:BATCH], in_=ags[l][a:b, :]
                        )
    nc.compile()
    return nc


def run(prep, trace=False):
    from concourse.bass_utils import run_bass_kernel_spmd

    nc = build_program(prep)
    in_maps = [
        {"xfull": prep.xpad, "idx": prep.idx16_all[k], "w": prep.w_all[k]}
        for k in range(prep.ncores)
    ]
    res = run_bass_kernel_spmd(
        nc, in_maps, core_ids=list(range(prep.ncores)), trace=trace
    )
    y_concat = np.concatenate(
        [res.results[k]["yout"] for k in range(prep.ncores)], axis=0
    )
    return y_concat[prep.perm], res


def kernel(x, weights, row, col):
    prep = _Prep(x, weights, row, col, N_NODES, NCORES, LAYERS)
    y, _ = run(prep, trace=False)
    return y



# revision 3
# speedup vs baseline: 5.6518x; 1.0516x over previous
"""Trainium2 Bass kernel for repeated sparse COO SpMM (GNN message passing).

y <- A @ y applied LAYERS times, A[row[e], col[e]] = weights[e].
N=100000 nodes, E=3200000 edges, B=16 features, 4 layers, 8 NeuronCores.

v4 strategy (1D partition by destination row):
  * Host: relabel nodes into 16 degree-snaked shards (core c owns shards
    c and c+8); per destination-tile (128 dests) the on-chip segment-sum
    is one strided DVE reduce with D = max degree in tile (degree-sorted
    tiles make the padding ~1%).
  * y is f32, packed 4 nodes per 256B row (dma_gather's stride quantum),
    so the whole graph fits ONE int16 gather window (25088 blocks). Each
    edge gathers the 256B block holding its source node; a host-built
    weight mask selects the node during the DVE multiply, and the
    strided reduce sums over D*4 sub-slots in f32.
  * dma_gather descriptor generation runs on one Q7 core pair per SWDGE
    queue; calls round-robin over all 4 queues so all 8 Q7 cores
    generate descriptors in parallel.
  * Each layer's AllGather is split in two (shards 0-7, then 8-15): the
    first fires mid-layer and overlaps the second half's gathers. The
    AllGather output IS the packed layout (same bytes), so the next
    layer gathers straight from it.
"""

import numpy as np

# ---------------------------------------------------------------- problem dims
N_NODES = 100000
N_EDGES = 3200000
BATCH = 16
LAYERS = 4
NCORES = 8
NSHARDS = 16
P = 128
PACK = 4  # nodes per 256B gather block (f32)
ROWF = PACK * BATCH  # 64 f32 per packed row

SLOT_BUDGET = 128  # msg-buffer slots per chunk (x256B per partition)
NUM_QUEUES = 4
CALL_SLOTS = 8  # slot-columns per dma_gather call (x128 = 1024 idxs)


def _mk_chunks(dt, t0, t1):
    chunks = []
    t = t0
    while t < t1:
        te = t
        acc = 0
        while te < t1 and (te == t or acc + dt[te] <= SLOT_BUDGET):
            acc += dt[te]
            te += 1
        chunks.append((t, te))
        t = te
    return chunks


class _Prep:
    """Host-side graph preprocessing, shared by kernel() and tests."""

    def __init__(self, x, weights, row, col, n_nodes, ncores, layers):
        n = n_nodes
        nps_real = n // NSHARDS  # real nodes per shard
        tiles_s = (nps_real + P - 1) // P  # tiles per shard
        nps = tiles_s * P  # padded shard size
        tiles = 2 * tiles_s  # tiles per core
        npc = 2 * nps  # positions per core
        npad = NSHARDS * nps
        nblocks = npad // PACK
        assert nblocks < 32768  # one int16 gather window

        row = np.asarray(row).astype(np.int64)
        col = np.asarray(col).astype(np.int64)
        weights = np.asarray(weights, dtype=np.float32)
        deg = np.bincount(row, minlength=n)

        # ascending-degree order, snake-assigned to 16 shards
        order = np.argsort(deg, kind="stable")
        blocks = order.reshape(nps_real, NSHARDS).copy()
        blocks[1::2] = blocks[1::2, ::-1]
        perm = np.empty(n, dtype=np.int64)
        for s in range(NSHARDS):
            perm[blocks[:, s]] = s * nps + np.arange(nps_real)

        new_row = perm[row]
        new_col = perm[col]

        # global position -> (core, local dest index)
        def to_core_local(p):
            v = p // nps
            return v % ncores, (v // ncores) * nps + p % nps

        # --- per-edge slot within its destination bucket -----------------
        eorder = np.argsort(new_row, kind="stable")
        sr = new_row[eorder]
        sc = new_col[eorder]
        sw_weights = weights[eorder]
        change = np.flatnonzero(np.diff(sr)) + 1
        starts = np.concatenate(([0], change))
        counts = np.diff(np.concatenate((starts, [len(sr)])))
        dests = sr[starts]
        j = np.arange(len(sr)) - np.repeat(starts, counts)  # rank in bucket

        dest_core, dest_local = to_core_local(dests)
        dest_ltile = dest_local // P
        e_core = np.repeat(dest_core, counts)
        e_ltile = np.repeat(dest_ltile, counts)
        e_p = np.repeat(dest_local, counts) % P

        dtc = np.ones((ncores, tiles), dtype=np.int64)
        np.maximum.at(dtc, (dest_core, dest_ltile), counts)
        # all cores share one program => tile widths must match across cores
        dt = dtc.max(axis=0)  # [tiles]
        off = np.concatenate(([0], np.cumsum(dt)[:-1]))  # col base per tile
        total_cols = int(dt.sum())

        # chunks per half (never straddle the shard boundary tile)
        chunks_a = _mk_chunks(dt, 0, tiles_s)
        chunks_b = _mk_chunks(dt, tiles_s, tiles)
        chunks = chunks_a + chunks_b
        chunk_cols = np.array([int(dt[a:b].sum()) for a, b in chunks])
        chunk_col_base = np.concatenate(([0], np.cumsum(chunk_cols)[:-1]))

        # --- per-edge slot column, weight mask, idx table ----------------
        e_slot = off[e_ltile] + j  # global slot column
        blk = sc // PACK
        sub = sc % PACK

        w8 = np.zeros((ncores, P, total_cols * PACK), dtype=np.float32)
        w8[e_core, e_p, e_slot * PACK + sub] = sw_weights

        # gather entry j -> partition j%128, free j//128 (slot col)
        e_entry = e_slot * P + e_p
        total_entries = total_cols * P
        flat_idx = np.zeros((ncores, total_entries), dtype=np.int16)
        flat_idx[e_core, e_entry] = blk.astype(np.int16)
        assert total_entries % 16 == 0
        wrapped = flat_idx.reshape(ncores, total_entries // 16, 16).transpose(
            0, 2, 1
        )
        idx16_all = np.ascontiguousarray(np.tile(wrapped, (1, 8, 1)))

        xp = np.zeros((npad, BATCH), dtype=np.float32)
        xp[perm] = np.asarray(x, dtype=np.float32)
        xpad = np.ascontiguousarray(xp.reshape(nblocks, ROWF))

        # unshard: node n -> y_concat[core*npc + local]
        pc, pl = to_core_local(perm)
        self.unshard = pc * npc + pl

        self.n_nodes = n
        self.ncores = ncores
        self.layers = layers
        self.tiles = tiles
        self.tiles_s = tiles_s
        self.nps = nps
        self.npc = npc
        self.npad = npad
        self.nblocks = nblocks
        self.dt = dt
        self.off = off
        self.chunks = chunks
        self.n_chunks_a = len(chunks_a)
        self.chunk_cols = chunk_cols
        self.chunk_col_base = chunk_col_base
        self.total_cols = total_cols
        self.total_entries = total_entries
        self.perm = perm
        self.w8 = w8  # f32 weight mask
        self.idx16_all = idx16_all
        self.xpad = xpad
        self.slots = int(total_cols) * P


_REG_CACHE = {}


def _dma_gather_raw(
    g, out_ap, in_ap, idxs_ap, num_idxs, elem_size, elem_step, queue_num
):
    """Non-transpose DRAM-source dma_gather without the (transpose-only)
    elem_size%256 restriction. Mirrors bass.BassGpSimd.dma_gather."""
    import concourse.mybir as mybir

    stride_bytes = elem_step * mybir.dt.size(in_ap.dtype)
    assert stride_bytes % 256 == 0
    _in_ap = g.lower_ap_dma(in_ap, for_custom_bir_dma=True)
    _idxs_ap = g.lower_ap(idxs_ap)
    _out_ap = g.lower_ap(out_ap)
    key = (id(g), num_idxs)
    if key not in _REG_CACHE:
        _REG_CACHE[key] = g.to_reg(num_idxs)
    return g.add_instruction(
        mybir.InstDMAGatherAnt(
            name=g.bass.get_next_instruction_name(),
            ins=[*_in_ap, _idxs_ap, g.lower_val_access(_REG_CACHE[key])],
            outs=[_out_ap],
            transpose=False,
            num_idxs=num_idxs,
            elem_size=elem_size,
            stride_bytes_256=stride_bytes // 256,
            gen_mode=0,
            single_packet=True,
            queue_num=queue_num,
            sbuf_tokens_per_rank=0,
            sbuf_free_dim_per_rank=0,
            sbuf_free_dim_pad_per_rank=0,
            sbuf_byte_offset=0,
        )
    )


def build_program(prep):
    import concourse.bass as bass
    import concourse.bacc as bacc
    import concourse.mybir as mybir
    import concourse.tile as tile

    ncores = prep.ncores
    npc = prep.npc
    nps = prep.nps
    npad = prep.npad
    layers = prep.layers
    ts = prep.tiles_s

    nc = bacc.Bacc(
        None,
        num_devices=ncores,
        num_swdge_queues=NUM_QUEUES,
    )
    f32 = mybir.dt.float32
    i16 = mybir.dt.int16
    xfull = nc.dram_tensor(
        "xfull", [prep.nblocks, ROWF], f32, kind="ExternalInput"
    )
    idx_d = nc.dram_tensor(
        "idx", [P, prep.total_entries // 16], i16, kind="ExternalInput"
    )
    w_d = nc.dram_tensor(
        "w", [P, prep.total_cols * PACK], f32, kind="ExternalInput"
    )
    yout = nc.dram_tensor("yout", [npc, BATCH], f32, kind="ExternalOutput")

    with tile.TileContext(nc) as tc:
        with (
            tc.tile_pool(name="res", bufs=1) as res_pool,
            tc.tile_pool(name="msgp", bufs=3) as msg_pool,
            tc.tile_pool(name="idxp", bufs=12) as idx_pool,
            tc.tile_pool(name="outp", bufs=1) as out_pool,
            tc.tile_pool(name="dram", bufs=1, space="DRAM") as dram_pool,
        ):
            w_s = res_pool.tile([P, prep.total_cols * PACK], f32, name="w_s")
            nc.sync.dma_start(out=w_s[:], in_=w_d[:])
            # prime the DVE dependency on the w_s load
            w_prime = res_pool.tile([P, 1], f32, name="w_prime")
            nc.vector.tensor_copy(out=w_prime[:], in_=w_s[:, 0:1])

            slices = [
                dram_pool.tile([npc, BATCH], f32, name=f"slice{i}")
                for i in range(max(layers - 1, 1))
            ]
            ags = [
                [
                    dram_pool.tile(
                        [npad // 2, BATCH],
                        f32,
                        addr_space="Shared",
                        name=f"ag{i}_{h}",
                    )
                    for h in range(2)
                ]
                for i in range(max(layers - 1, 1))
            ]
            ybufs = [
                dram_pool.tile([npad, BATCH], f32, name=f"ybuf{i}")
                for i in range(max(layers - 1, 1))
            ]

            qn = 0
            for l in range(layers):
                if l == 0:
                    src = xfull[:, :]
                else:
                    src = ybufs[l - 1][:, :].rearrange(
                        "(n k) b -> n (k b)", k=PACK
                    )
                dst = yout if l == layers - 1 else slices[l]
                ylayer = out_pool.tile(
                    [P, prep.tiles * BATCH], f32, name=f"ylayer{l}", tag="yl"
                )

                def emit_half(h):
                    nonlocal qn
                    lo = 0 if h == 0 else prep.n_chunks_a
                    hi = prep.n_chunks_a if h == 0 else len(prep.chunks)
                    for ci in range(lo, hi):
                        t0, t1 = prep.chunks[ci]
                        ccols = int(prep.chunk_cols[ci])
                        cb = int(prep.chunk_col_base[ci])
                        msg = msg_pool.tile(
                            [P, ccols * ROWF], f32, name="msg", tag="msg"
                        )
                        for s0 in range(0, ccols, CALL_SLOTS):
                            sw = min(CALL_SLOTS, ccols - s0)
                            n_idx = P * sw
                            eb = (cb + s0) * P
                            idxt = idx_pool.tile(
                                [P, n_idx // 16], i16, name="idxt", tag="idxt"
                            )
                            nc.sync.dma_start(
                                out=idxt[:],
                                in_=idx_d[:, eb // 16 : (eb + n_idx) // 16],
                            )
                            _dma_gather_raw(
                                nc.gpsimd,
                                out_ap=msg[
                                    :, s0 * ROWF : (s0 + sw) * ROWF
                                ].rearrange("p (c f) -> p c f", f=ROWF),
                                in_ap=src,
                                idxs_ap=idxt[:],
                                num_idxs=n_idx,
                                elem_size=ROWF,
                                elem_step=ROWF,
                                queue_num=qn,
                            )
                            qn = (qn + 1) % NUM_QUEUES
                        nc.vector.tensor_tensor(
                            out=msg[:].rearrange("p (d f) -> p d f", f=BATCH),
                            in0=msg[:].rearrange("p (d f) -> p d f", f=BATCH),
                            in1=w_s[
                                :, cb * PACK : (cb + ccols) * PACK
                            ].to_broadcast([P, ccols * PACK, BATCH]),
                            op=mybir.AluOpType.mult,
                        )
                        for t in range(t0, t1):
                            d_t = int(prep.dt[t])
                            o = (int(prep.off[t]) - cb) * ROWF
                            base2 = msg[:, o : o + BATCH]
                            in_ap = bass.AP(
                                base2.tensor,
                                base2.offset,
                                [
                                    base2.ap[0],
                                    [1, BATCH],
                                    [BATCH, d_t * PACK],
                                ],
                            )
                            nc.vector.tensor_reduce(
                                out=ylayer[:, t * BATCH : (t + 1) * BATCH],
                                in_=in_ap,
                                axis=mybir.AxisListType.X,
                                op=mybir.AluOpType.add,
                            )
                    # write this half's slice and (if not last layer) AllGather
                    tcol0 = 0 if h == 0 else ts * BATCH
                    tcol1 = ts * BATCH if h == 0 else prep.tiles * BATCH
                    r0 = 0 if h == 0 else nps
                    r1 = nps if h == 0 else npc
                    nc.sync.dma_start(
                        out=dst[r0:r1, :].rearrange("(t p) f -> p t f", p=P),
                        in_=ylayer[:, tcol0:tcol1].rearrange(
                            "p (t f) -> p t f", f=BATCH
                        ),
                    )
                    if l < layers - 1:
                        nc.gpsimd.collective_compute(
                            "AllGather",
                            mybir.AluOpType.bypass,
                            replica_groups=[list(range(ncores))],
                            ins=[dst[r0:r1, :]],
                            outs=[ags[l][h][:, :]],
                        )
                        nc.sync.dma_start(
                            out=ybufs[l][
                                h * (npad // 2) : (h + 1) * (npad // 2), :
                            ],
                            in_=ags[l][h][:, :],
                        )

                emit_half(0)
                emit_half(1)
    nc.compile()
    return nc


def run(prep, trace=False):
    from concourse.bass_utils import run_bass_kernel_spmd

    nc = build_program(prep)
    in_maps = [
        {"xfull": prep.xpad, "idx": prep.idx16_all[k], "w": prep.w8[k]}
        for k in range(prep.ncores)
    ]
    res = run_bass_kernel_spmd(
        nc, in_maps, core_ids=list(range(prep.ncores)), trace=trace
    )
    y_concat = np.concatenate(
        [res.results[k]["yout"] for k in range(prep.ncores)], axis=0
    )
    return y_concat[prep.unshard], res


def kernel(x, weights, row, col):
    prep = _Prep(x, weights, row, col, N_NODES, NCORES, LAYERS)
    y, _ = run(prep, trace=False)
    return y
